# revision 1
# baseline (speedup 1.0000x reference)
"""Segment-mean + linear head kernel for TRN2 (8 NeuronCores, data parallel).

Reference computation (per batch row r):
    seg-mean of x[r] over tokens sharing word_id, gathered back per token,
    then linear head W,b:  logits[r,s,:] = mean_{s': wid[s']=wid[s]} x[r,s'] @ W.T + b

Key identity: the mean and the linear head commute, so
    logits[r,s,:] = Z[wid[s],:]  with  Z[g,:] = (sum_{s in g} y[s,:]) / max(cnt_g,1) + b,
    y = x @ W.T   ([S,15] -- tiny channel dim).
The segment scatter/gather is done with 0/1 indicator matmuls on the tensor
engine; indicators are generated on-chip with iota + is_equal compares.
Word ids are sorted per row, so each 128-wide segment chunk only touches a
few contiguous 128-token tiles; that schedule is computed on the host from
the actual ids (union across cores so the SPMD program is identical).
"""

import sys
from contextlib import ExitStack

import numpy as np

for _p in ("/opt/trn_rl_repo",):
    if _p not in sys.path:
        sys.path.insert(0, _p)

import concourse.bass as bass
import concourse.bacc as bacc
import concourse.tile as tile
from concourse import mybir
from concourse.bass_utils import run_bass_kernel_spmd

B, S, H, C = 16, 2048, 1024, 15
NW = 800
NCORES = 8
RPC = B // NCORES          # rows per core
T = S // 128               # 128-token tiles per row
NK = H // 128              # 128-wide h chunks
NCHUNK = (NW + 127) // 128 # 128-wide segment chunks

F32 = mybir.dt.float32
F32R = mybir.dt.float32r
BF16 = mybir.dt.bfloat16
I32 = mybir.dt.int32
EQ = mybir.AluOpType.is_equal
MULT = mybir.AluOpType.mult


def _schedule(word_ids):
    """chunks_t[lr][t]: sorted segment-chunk ids present in tile t of local row
    lr on ANY core; windows[lr][j]: sorted tiles where chunk j is active."""
    cid = (np.asarray(word_ids).astype(np.int64) // 128).reshape(B, T, 128)
    chunks_t = [[set() for _ in range(T)] for _ in range(RPC)]
    for core in range(NCORES):
        for lr in range(RPC):
            g = core * RPC + lr
            for t in range(T):
                for j in np.unique(cid[g, t]):
                    chunks_t[lr][t].add(int(j))
    chunks_t = [[sorted(s) for s in row] for row in chunks_t]
    windows = [
        [[t for t in range(T) if j in chunks_t[lr][t]] for j in range(NCHUNK)]
        for lr in range(RPC)
    ]
    return chunks_t, windows


def _build(chunks_t, windows):
    nc = bacc.Bacc("TRN2", target_bir_lowering=False, debug=False)
    x_d = nc.declare_dram_parameter("x", [RPC, S, H], BF16, isOutput=False)
    widr_d = nc.declare_dram_parameter("widr", [RPC, S], F32R, isOutput=False)
    widc_d = nc.declare_dram_parameter("widc", [RPC, 128, T], F32, isOutput=False)
    wt_d = nc.declare_dram_parameter("wt", [NK, 128, C], BF16, isOutput=False)
    b_d = nc.declare_dram_parameter("bias", [1, 16], F32R, isOutput=False)
    out_d = nc.declare_dram_parameter("out", [RPC, 128, T * C], F32, isOutput=True)

    with tile.TileContext(nc) as tc, ExitStack() as ctx:
        consts = ctx.enter_context(tc.tile_pool(name="consts", bufs=1))
        widp = ctx.enter_context(tc.tile_pool(name="widp", bufs=2))
        xpool = ctx.enter_context(tc.tile_pool(name="xpool", bufs=3))
        xtpool = ctx.enter_context(tc.tile_pool(name="xtpool", bufs=2))
        ytsb = ctx.enter_context(tc.tile_pool(name="ytsb", bufs=2))
        y1p = ctx.enter_context(tc.tile_pool(name="y1p", bufs=4))
        apool = ctx.enter_context(tc.tile_pool(name="apool", bufs=4))
        zpool = ctx.enter_context(tc.tile_pool(name="zpool", bufs=2))
        scp = ctx.enter_context(tc.tile_pool(name="scp", bufs=4))
        opool = ctx.enter_context(tc.tile_pool(name="opool", bufs=2))
        tpps = ctx.enter_context(tc.tile_pool(name="tpps", bufs=3, space="PSUM"))
        ypps = ctx.enter_context(tc.tile_pool(name="ypps", bufs=2, space="PSUM"))
        smps = ctx.enter_context(tc.tile_pool(name="smps", bufs=2, space="PSUM"))

        # --- constants ---
        iotag = consts.tile([128, NCHUNK, 128], F32, tag="iotag")
        nc.gpsimd.iota(iotag[:], [[128, NCHUNK], [1, 128]], channel_multiplier=0,
                       allow_small_or_imprecise_dtypes=True)
        pidx = consts.tile([128, NCHUNK], F32, tag="pidx")
        nc.gpsimd.iota(pidx[:], [[128, NCHUNK]], channel_multiplier=1,
                       allow_small_or_imprecise_dtypes=True)
        i0 = consts.tile([128, 128], F32, tag="i0")
        nc.gpsimd.iota(i0[:], [[1, 128]], channel_multiplier=0,
                       allow_small_or_imprecise_dtypes=True)
        p0 = consts.tile([128, 1], F32, tag="p0")
        nc.gpsimd.iota(p0[:], [[0, 1]], channel_multiplier=1,
                       allow_small_or_imprecise_dtypes=True)
        ident = consts.tile([128, 128], F32, tag="ident")
        nc.vector.tensor_scalar(ident[:], i0[:], p0[:], None, op0=EQ)
        ident_bf = consts.tile([128, 128], BF16, tag="identbf")
        nc.vector.tensor_scalar(ident_bf[:], i0[:], p0[:], None, op0=EQ)
        wt_sb = consts.tile([128, NK, C], BF16, tag="wt")
        nc.sync.dma_start(wt_sb[:], wt_d.rearrange("k h c -> h k c"))
        b_sb = consts.tile([1, 16], F32R, tag="bias")
        nc.sync.dma_start(b_sb[:], b_d[:])
        ones_col = consts.tile([1, 128], F32R, tag="ones")
        nc.vector.memset(ones_col[:].bitcast(F32), 1.0)
        b_bc = consts.tile([128, 16], BF16, tag="bbc")
        bb_ps = smps.tile([128, 16], F32, tag="sm")
        nc.tensor.matmul(bb_ps[:], ones_col[:], b_sb[:], start=True, stop=True)
        nc.any.tensor_copy(b_bc[:], bb_ps[:])

        for r in range(RPC):
            ct = chunks_t[r]
            win = windows[r]
            present = [j for j in range(NCHUNK) if win[j]]

            widr_sb = widp.tile([1, S], F32R, tag="widr")
            nc.sync.dma_start(widr_sb[:], widr_d[r : r + 1, :])
            widc_sb = widp.tile([128, T], F32, tag="widc")
            nc.sync.dma_start(widc_sb[:], widc_d[r])
            xr = x_d[r].rearrange("(t p) h -> p t h", p=128)
            wid_bc = widp.tile([128, S], F32, tag="widbc")
            for q in range(S // 512):
                wb_ps = tpps.tile([128, 512], F32, tag="tp")
                nc.tensor.matmul(
                    wb_ps[:],
                    ones_col[:],
                    widr_sb[0:1, 512 * q : 512 * q + 512],
                    start=True,
                    stop=True,
                )
                nc.any.tensor_copy(wid_bc[:, 512 * q : 512 * q + 512], wb_ps[:])

            sums_sb = zpool.tile([128, NCHUNK, 16], F32, tag="sums")
            nc.vector.memset(sums_sb[:], 0.0)
            # --- pass 1: y = x@W.T per token, scatter into segment sums ---
            for g4 in range(T // 4):
                x_sb = xpool.tile([128, 4, H], BF16)
                nc.sync.dma_start(x_sb[:], xr[:, 4 * g4 : 4 * g4 + 4, :])
                xt_sb = xtpool.tile([128, NK, 512], BF16)
                for ti in range(4):
                    for half in range(2):
                        tp = tpps.tile([128, 512], BF16, tag="tp")
                        for kk in range(4):
                            k = 4 * half + kk
                            nc.tensor.transpose(
                                tp[:, 128 * kk : 128 * kk + 128],
                                x_sb[:, ti, 128 * k : 128 * k + 128],
                                ident_bf[:],
                            )
                        nc.any.tensor_copy(
                            xt_sb[:, 4 * half : 4 * half + 4, 128 * ti : 128 * ti + 128],
                            tp[:].rearrange("p (k s) -> p k s", k=4),
                        )
                yp = ypps.tile([C, 512], F32)
                for k in range(NK):
                    nc.tensor.matmul(
                        yp[:],
                        wt_sb[:, k, :],
                        xt_sb[:, k, :],
                        start=(k == 0),
                        stop=(k == NK - 1),
                    )
                yt = ytsb.tile([C, 512], BF16)
                nc.any.tensor_copy(yt[:], yp[:])
                for ti in range(4):
                    t = 4 * g4 + ti
                    ytp = smps.tile([128, 16], BF16, tag="sm")
                    nc.tensor.transpose(
                        ytp[:, 0:C],
                        yt[:, 128 * ti : 128 * ti + 128],
                        ident_bf[:C, :C],
                    )
                    y1 = y1p.tile([128, 16], BF16)
                    nc.any.tensor_copy(y1[:, 0:C], ytp[:, 0:C])
                    nc.vector.memset(y1[:, C : C + 1], 1.0)
                    for j in ct[t]:
                        a = apool.tile([128, 128], BF16, tag="a")
                        nc.vector.tensor_scalar(
                            a[:], iotag[:, j, :], widc_sb[:, t : t + 1], None, op0=EQ
                        )
                        part = smps.tile([128, 16], F32, tag="sm")
                        nc.tensor.matmul(
                            part[:],
                            a[:],
                            y1[:],
                            start=True,
                            stop=True,
                        )
                        nc.vector.tensor_add(
                            sums_sb[:, j, :], sums_sb[:, j, :], part[:]
                        )

            # --- Z: means + bias per segment chunk ---
            z_sb = zpool.tile([128, NCHUNK, 16], BF16, tag="z")
            for j in present:
                cm = scp.tile([128, 1], F32, tag="cm")
                nc.vector.tensor_scalar_max(cm[:], sums_sb[:, j, C : C + 1], 1.0)
                rc = scp.tile([128, 1], F32, tag="rc")
                nc.vector.reciprocal(rc[:], cm[:])
                nc.vector.tensor_scalar(
                    z_sb[:, j, :], sums_sb[:, j, :], rc[:], None, op0=MULT
                )
                nc.vector.tensor_add(z_sb[:, j, :], z_sb[:, j, :], b_bc[:])

            # --- pass 2: gather Z back to tokens ---
            orow = opool.tile([128, T * C], F32)
            for t in range(T):
                ops_ = smps.tile([128, 16], F32, tag="sm")
                cl = ct[t]
                for idx, j in enumerate(cl):
                    at = apool.tile([128, 128], BF16, tag="a")
                    nc.vector.tensor_scalar(
                        at[:],
                        wid_bc[:, 128 * t : 128 * t + 128],
                        pidx[:, j : j + 1],
                        None,
                        op0=EQ,
                    )
                    nc.tensor.matmul(
                        ops_[:],
                        at[:],
                        z_sb[:, j, :],
                        start=(idx == 0),
                        stop=(idx == len(cl) - 1),
                    )
                nc.any.tensor_copy(orow[:, C * t : C * t + C], ops_[:, 0:C])
            nc.sync.dma_start(out_d[r], orow[:])

    nc.compile()
    return nc


def _prep_host(x, word_ids, W, b):
    import ml_dtypes
    wid32 = np.ascontiguousarray(np.asarray(word_ids).astype(np.int64))
    widf = wid32.astype(np.float32)
    widc = np.ascontiguousarray(
        widf.reshape(B, T, 128).transpose(0, 2, 1)
    )  # [B,128,T]
    wtk = np.ascontiguousarray(
        np.asarray(W, dtype=np.float32).T.reshape(NK, 128, C)
    ).astype(ml_dtypes.bfloat16)
    bp = np.zeros((1, 16), dtype=np.float32)
    bp[0, :C] = np.asarray(b, dtype=np.float32)
    return wid32, widf, widc, wtk, bp


def _run(x, word_ids, W, b, **spmd_kwargs):
    import ml_dtypes
    x = np.ascontiguousarray(np.asarray(x, dtype=np.float32)).astype(ml_dtypes.bfloat16)
    wid32, widf, widc, wtk, bp = _prep_host(x, word_ids, W, b)
    chunks_t, windows = _schedule(wid32)
    nc = _build(chunks_t, windows)

    in_maps = []
    for core in range(NCORES):
        r0 = core * RPC
        in_maps.append(
            {
                "x": x[r0 : r0 + RPC],
                "widr": widf[r0 : r0 + RPC],
                "widc": widc[r0 : r0 + RPC],
                "wt": wtk,
                "bias": bp,
            }
        )
    res = run_bass_kernel_spmd(nc, in_maps, list(range(NCORES)), **spmd_kwargs)
    outs = []
    for core in range(NCORES):
        o = res.results[core]["out"]  # [RPC, 128, T*C]
        o = o.reshape(RPC, 128, T, C).transpose(0, 2, 1, 3).reshape(RPC, S, C)
        outs.append(o)
    full = np.ascontiguousarray(np.concatenate(outs, axis=0).astype(np.float32))
    return full, res


def kernel(x, word_ids, W, b):
    return _run(x, word_ids, W, b)[0]


if __name__ == "__main__":
    rng = np.random.default_rng(0)
    x = rng.standard_normal((B, S, H), dtype=np.float32)
    wid = np.sort(rng.integers(0, NW, (B, S)), axis=-1)
    W = rng.standard_normal((C, H), dtype=np.float32) / np.sqrt(H)
    b = np.zeros((C,), dtype=np.float32)
    out = kernel(x, wid, W, b)
    print(out.shape, out.dtype)

